# revision 1
# baseline (speedup 1.0000x reference)
"""BiLSTM-CRF Trainium2 kernel.

Sharding: 8 cores = 2 directions x 4 batch-groups of 8 examples.
Each core runs the same Bass program on different data:
  phase 1: P = X @ Wih.T + b      (parallel GEMM over all tokens -> DRAM)
  phase 2: LSTM scan over S steps (PE: h.T stationary, Whh.T streaming;
           ACT: gate nonlinearities; DVE: cell update; PE-transpose h)
  phase 3: feats_half = H_seq @ W_out_half.T
Host: embedding gather, time-reversal for the backward direction,
summing the two feature halves + b_out, Viterbi DP + backtrace.
"""

import numpy as np
from contextlib import ExitStack

import concourse.bass as bass
from concourse import bacc
import concourse.mybir as mybir
from concourse import tile
from concourse.bass_utils import run_bass_kernel_spmd

F32 = mybir.dt.float32
AF = mybir.ActivationFunctionType

B, S, E, H, T = 32, 512, 512, 512, 16
G4 = 4 * H          # 2048 gates
NCORES = 8
NGRP = 4            # batch groups
BL = B // NGRP      # 8 examples per core


def build_program(nc, s_len=S, bl=BL):
    toks = bl * s_len
    assert s_len % 128 == 0
    xt = nc.declare_dram_parameter("xt", [E, toks], F32, isOutput=False)
    wih = nc.declare_dram_parameter("wih", [E, G4], F32, isOutput=False)
    whh = nc.declare_dram_parameter("whh", [H, G4], F32, isOutput=False)
    bb = nc.declare_dram_parameter("bb", [128, G4], F32, isOutput=False)
    wo = nc.declare_dram_parameter("wo", [H, T], F32, isOutput=False)
    ident = nc.declare_dram_parameter("ident", [bl, bl], F32, isOutput=False)
    feats = nc.declare_dram_parameter("feats", [bl, T, s_len], F32, isOutput=True)
    pd = nc.dram_tensor("pscr", [bl, s_len, G4], F32)

    KE = E // 128
    KH = H // 128
    NT = G4 // 512
    MT = toks // 128

    with tile.TileContext(nc) as tc, ExitStack() as ctx:
        wpool = ctx.enter_context(tc.tile_pool(name="persist", bufs=1))
        whh_sb = wpool.tile([128, KH, G4], F32, tag="whh")
        nc.sync.dma_start(whh_sb[:], whh.rearrange("(k p) n -> p k n", p=128))
        wo_sb = wpool.tile([128, KH, T], F32, tag="wo")
        nc.sync.dma_start(wo_sb[:], wo.rearrange("(k p) n -> p k n", p=128))
        id_sb = wpool.tile([bl, bl], F32, tag="id")
        nc.sync.dma_start(id_sb[:], ident[:])
        bb_sb = wpool.tile([128, G4], F32, tag="bb")
        nc.sync.dma_start(bb_sb[:], bb[:])
        # h.T history: [p, k, b, s+1]; slot 0 is h_init = 0, step t writes slot t+1
        ht = wpool.tile([128, KH, bl, s_len], F32, tag="ht")  # h.T history (h_0..h_{S-1})
        htc = wpool.tile([128, KH, bl], F32, tag="htc")  # current h.T (static addr for LDW)
        c_sb = wpool.tile([bl, H], F32, tag="c")
        nc.gpsimd.memset(htc[:], 0.0)
        nc.gpsimd.memset(c_sb[:], 0.0)

        # ---- phase 1: P = X @ Wih.T + b over 128-token tiles ----
        with tc.tile_pool(name="xtl", bufs=3) as xp, \
             tc.tile_pool(name="p1ps", bufs=4, space="PSUM") as pp, \
             tc.tile_pool(name="wihp", bufs=1) as wihp, \
             tc.tile_pool(name="pout", bufs=4) as pop:
            wih_sb = wihp.tile([128, KE, G4], F32)
            nc.sync.dma_start(wih_sb[:], wih.rearrange("(k p) n -> p k n", p=128))
            xtr = xt.rearrange("(k p) n -> p k n", p=128)
            for m in range(MT):
                xt_sb = xp.tile([128, KE, 128], F32)
                nc.sync.dma_start(xt_sb[:], xtr[:, :, m * 128:(m + 1) * 128])
                bidx, s0 = divmod(m * 128, s_len)
                for n in range(NT):
                    ps = pp.tile([128, 512], F32)
                    for k in range(KE):
                        nc.tensor.matmul(
                            ps[:], xt_sb[:, k, :],
                            wih_sb[:, k, n * 512:(n + 1) * 512],
                            start=(k == 0), stop=(k == KE - 1))
                    po = pop.tile([128, 512], F32)
                    nc.vector.tensor_add(po[:], ps[:], bb_sb[:, n * 512:(n + 1) * 512])
                    nc.sync.dma_start(pd[bidx, s0:s0 + 128, n * 512:(n + 1) * 512], po[:])

        # ---- phase 2: sequential scan (fully static unroll) ----
        with tc.tile_pool(name="ptl", bufs=4) as ptp, \
             tc.tile_pool(name="gsb", bufs=2) as gp, \
             tc.tile_pool(name="gps", bufs=1, space="PSUM") as gpsp, \
             tc.tile_pool(name="tps", bufs=2, space="PSUM") as tpsp:
            for t in range(s_len):
                pt_sb = ptp.tile([bl, G4], F32, tag="pt")
                nc.sync.dma_start(pt_sb[:], pd[:, t, :])
                ps = gpsp.tile([bl, G4], F32, tag="gpsum")
                for n in range(NT):
                    for k in range(KH):
                        lhs = htc[:, k, :] if t == 0 else ht[:, k, :, t - 1]
                        nc.tensor.matmul(
                            ps[:, n * 512:(n + 1) * 512], lhs,
                            whh_sb[:, k, n * 512:(n + 1) * 512],
                            start=(k == 0), stop=(k == KH - 1))
                gsb = gp.tile([bl, G4], F32, tag="gates")
                nc.vector.tensor_add(gsb[:], ps[:], pt_sb[:])
                act = gp.tile([bl, G4], F32, tag="act")
                nc.scalar.activation(act[:, 0:1024], gsb[:, 0:1024], AF.Sigmoid)
                nc.scalar.activation(act[:, 1024:1536], gsb[:, 1024:1536], AF.Tanh)
                nc.scalar.activation(act[:, 1536:2048], gsb[:, 1536:2048], AF.Sigmoid)
                a_i, a_f = act[:, 0:512], act[:, 512:1024]
                a_g, a_o = act[:, 1024:1536], act[:, 1536:2048]
                t1 = gp.tile([bl, 512], F32, tag="t1")
                nc.vector.tensor_mul(t1[:], a_i, a_g)
                nc.vector.tensor_mul(c_sb[:], c_sb[:], a_f)
                nc.vector.tensor_add(c_sb[:], c_sb[:], t1[:])
                tch = gp.tile([bl, 512], F32, tag="tch")
                nc.scalar.activation(tch[:], c_sb[:], AF.Tanh)
                h_sb = gp.tile([bl, 512], F32, tag="h")
                nc.vector.tensor_mul(h_sb[:], a_o, tch[:])
                tp = tpsp.tile([128, KH, bl, 1], F32, tag="tpsum")
                for k in range(KH):
                    nc.tensor.transpose(tp[:, k, :, 0], h_sb[:, k * 128:(k + 1) * 128], id_sb[:])
                nc.scalar.copy(ht[:, :, :, t:t + 1], tp[:])

        # ---- phase 3: feats_half.T = WoT.T @ H.T ----
        with tc.tile_pool(name="f3", bufs=2) as f3p, \
             tc.tile_pool(name="f3ps", bufs=2, space="PSUM") as f3ps:
            for bi in range(bl):
                ps = f3ps.tile([T, s_len], F32)
                for k in range(KH):
                    nc.tensor.matmul(ps[:], wo_sb[:, k, :], ht[:, k, bi, :],
                                     start=(k == 0), stop=(k == KH - 1))
                fo = f3p.tile([T, s_len], F32)
                nc.vector.tensor_copy(fo[:], ps[:])
                nc.sync.dma_start(feats[bi], fo[:])
    return nc


_NC_CACHE = {}


def _get_nc():
    if "nc" not in _NC_CACHE:
        nc = bacc.Bacc("TRN2")
        build_program(nc)
        nc.finalize()
        _NC_CACHE["nc"] = nc
    return _NC_CACHE["nc"]


def make_in_maps(emb, Wih_f, Whh_f, b_f, Wih_b, Whh_b, b_b, W_out, s_len=S, bl=BL):
    """emb: [B, s_len, E] float32. Returns 8 per-core input maps."""
    in_maps = []
    for c in range(NCORES):
        d, g = divmod(c, NGRP)
        x = emb[g * bl:(g + 1) * bl]
        if d == 1:
            x = x[:, ::-1]
        xtm = np.ascontiguousarray(x.reshape(bl * s_len, E).T).astype(np.float32)
        Wih, Whh, bvec = (Wih_f, Whh_f, b_f) if d == 0 else (Wih_b, Whh_b, b_b)
        wo_half = W_out[:, :H] if d == 0 else W_out[:, H:]
        in_maps.append({
            "xt": xtm,
            "wih": np.ascontiguousarray(np.asarray(Wih, np.float32).T),
            "whh": np.ascontiguousarray(np.asarray(Whh, np.float32).T),
            "bb": np.tile(np.asarray(bvec, np.float32)[None, :], (128, 1)),
            "wo": np.ascontiguousarray(np.asarray(wo_half, np.float32).T),
            "ident": np.eye(bl, dtype=np.float32),
        })
    return in_maps


def assemble_feats(results, b_out, s_len=S, bl=BL):
    feats = np.zeros((NGRP * bl, s_len, T), np.float32)
    for c in range(NCORES):
        d, g = divmod(c, NGRP)
        f = np.transpose(np.asarray(results[c]["feats"]), (0, 2, 1))  # [bl, s, T]
        if d == 1:
            f = f[:, ::-1]
        feats[g * bl:(g + 1) * bl] += f
    feats += np.asarray(b_out, np.float32)[None, None, :]
    return feats


def viterbi(feats, trans, start, stop):
    Bq, Sq, Tq = feats.shape
    v = feats[:, 0] + start[None, :]
    idxs = np.zeros((Sq - 1, Bq, Tq), np.int32)
    for s in range(1, Sq):
        scores = v[:, :, None] + trans[None, :, :]
        idxs[s - 1] = np.argmax(scores, axis=1)
        v = np.max(scores, axis=1) + feats[:, s]
    last = np.argmax(v + stop[None, :], axis=-1).astype(np.int32)
    tags = np.zeros((Bq, Sq), np.int32)
    tags[:, -1] = last
    cur = last
    for s in range(Sq - 2, -1, -1):
        cur = idxs[s][np.arange(Bq), cur].astype(np.int32)
        tags[:, s] = cur
    return tags


def kernel(sentence, embedding, Wih_f, Whh_f, b_f, Wih_b, Whh_b, b_b,
           W_out, b_out, transitions, start_trans, stop_trans):
    sentence = np.asarray(sentence)
    emb = np.asarray(embedding, np.float32)[sentence.astype(np.int64)]  # [B, S, E]
    nc = _get_nc()
    in_maps = make_in_maps(emb, np.asarray(Wih_f), np.asarray(Whh_f), np.asarray(b_f),
                           np.asarray(Wih_b), np.asarray(Whh_b), np.asarray(b_b),
                           np.asarray(W_out))
    res = run_bass_kernel_spmd(nc, in_maps, list(range(NCORES))).results
    feats = assemble_feats(res, np.asarray(b_out))
    return viterbi(feats, np.asarray(transitions, np.float32),
                   np.asarray(start_trans, np.float32),
                   np.asarray(stop_trans, np.float32))



# revision 8
# speedup vs baseline: 2.0437x; 2.0437x over previous
"""BiLSTM-CRF Trainium2 kernel, v3.

Sharding: 8 cores x (4 examples, BOTH directions) = batch 32, data-parallel.
Each core runs two independent LSTM scans (fwd + bwd over its 4 examples),
interleaved so PE matmuls of one direction overlap the elementwise chain of
the other.

Precision plan (Viterbi tag flips are margin-limited, so matmul paths need
care): phase 1 (input projection) runs in fp32 and stores p as a bf16 hi+lo
pair; the recurrent matmul runs in fp16 (h and Whh); the h history is kept
as an fp16 hi + bf16 residual pair so phase 3 (features) can run in exact
fp32.  Emulated on host: 3/16384 mismatched tags.

Per-step layout: gate pre-activations in PSUM as
  [partition row = 32*j + b,  512 cols = (i|f|o|g) each 128, hidden slice j]
via 4 column-tiled concurrent matmuls (tile_position=(0,32j)) per k-chunk
(fp16 stationary hbar[k]=2h^T, fp16 moving Whh chunks), then 4 diagonal
(32j,32j) matmuls add p: stationary [I4;I4] stacked so one N=512 bf16 matmul
adds p_hi+p_lo exactly.  The elementwise chain uses only tanh (sigmoid via
tanh identity; g-columns pre-doubled on host, h kept as 2h with Whh/W_out
pre-halved) on [*,128]-col tiles.  h^T for the next step is produced by
gathering the 4 h2 strips to partitions 0..3 (gpsimd copies) and running 4
small fp32 matmuls against I4 at tile (0,0) (row-group!=0 transposes fault
on this HW).  Host: embedding gather, weight permute/scale, Viterbi DP.
"""

import numpy as np
from contextlib import ExitStack

import ml_dtypes
import concourse.bass as bass
from concourse import bacc
import concourse.mybir as mybir
from concourse import tile
from concourse.bass_utils import run_bass_kernel_spmd

F32 = mybir.dt.float32
F16 = mybir.dt.float16
BF16 = mybir.dt.bfloat16
AF = mybir.ActivationFunctionType
ALU = mybir.AluOpType
BF16NP = ml_dtypes.bfloat16

B, S, E, H, T = 32, 512, 512, 512, 16
G4 = 4 * H           # 2048 gate pre-activations per direction
NCORES = 8
BL = B // NCORES     # 4 examples per core
ND = 2               # directions per core
KH = H // 128        # 4 hidden k-chunks
KE = E // 128        # 4 embedding k-chunks
NJ = 4               # hidden slices / psum column-tile groups
MT = (BL * S) // 128  # 16 token tiles per direction (128 tokens each)
TPM = 128 // BL      # 32 timesteps per token tile


def build_program(nc):
    xts = [nc.declare_dram_parameter(f"xt{d}", [E, BL * S], F32, isOutput=False)
           for d in range(ND)]
    wih = nc.declare_dram_parameter("wih", [ND, E, G4], F32, isOutput=False)
    whh = nc.declare_dram_parameter("whh", [ND, H, G4], F16, isOutput=False)
    bb = nc.declare_dram_parameter("bb", [ND, 128, NJ, 512], F32, isOutput=False)
    wo = nc.declare_dram_parameter("wo", [ND, KH, 128, T], F32, isOutput=False)
    identp = nc.declare_dram_parameter("identp", [128, BL], BF16, isOutput=False)
    identt = nc.declare_dram_parameter("identt", [BL, BL], F32, isOutput=False)
    feats = nc.declare_dram_parameter("feats", [ND, BL, T, S], F32, isOutput=True)
    pD = nc.dram_tensor("pscr", [ND, MT, NJ, 2, 128, 512], BF16)

    with tile.TileContext(nc) as tc, ExitStack() as ctx:
        wpool = ctx.enter_context(tc.tile_pool(name="persist", bufs=1))
        whh_sb = wpool.tile([128, ND, KH, G4], F16, tag="whh")
        for d in range(ND):
            nc.sync.dma_start(whh_sb[:, d], whh[d].rearrange("(k p) n -> p k n", p=128))
        idp_sb = wpool.tile([128, BL], BF16, tag="idp")
        nc.sync.dma_start(idp_sb[:], identp[:])
        idt_sb = wpool.tile([BL, BL], F32, tag="idt")
        nc.sync.dma_start(idt_sb[:], identt[:])
        # hbar history (hi fp16 + lo bf16): [128, d, k, b, t+1]; slot 0 = 0
        hTh = wpool.tile([128, ND, KH, BL, S + 1], F16, tag="hTh")
        hTl = wpool.tile([128, ND, KH, BL, S + 1], BF16, tag="hTl")
        nc.gpsimd.memset(hTh[:, :, :, :, 0], 0.0)
        nc.gpsimd.memset(hTl[:, :, :, :, 0], 0.0)
        c2 = wpool.tile([128, ND, 128], F32, tag="c2")  # cell state as 2c
        nc.vector.memset(c2[:], 0.0)

        # ---- phase 1 (fp32): P = X @ Wih_perm + b -> bf16 hi/lo in DRAM ----
        with tc.tile_pool(name="p1x", bufs=3) as xp, \
             tc.tile_pool(name="p1ps", bufs=4, space="PSUM") as pp, \
             tc.tile_pool(name="p1w", bufs=1) as wp, \
             tc.tile_pool(name="p1o", bufs=3) as pop:
            bb_sb = wp.tile([128, ND, NJ, 512], F32, tag="bb")
            nc.sync.dma_start(bb_sb[:], bb.rearrange("d p j n -> p d j n"))
            for d in range(ND):
                wih_sb = wp.tile([128, KE, G4], F32, tag="wih")
                nc.sync.dma_start(wih_sb[:], wih[d].rearrange("(k p) n -> p k n", p=128))
                xtr = xts[d].rearrange("(k p) n -> p k n", p=128)
                for m in range(MT):
                    xt_sb = xp.tile([128, KE, 128], F32, tag="xt")
                    nc.sync.dma_start(xt_sb[:], xtr[:, :, m * 128:(m + 1) * 128])
                    for j in range(NJ):
                        ps = pp.tile([128, 512], F32, tag="ps")
                        for k in range(KE):
                            nc.tensor.matmul(
                                ps[:], xt_sb[:, k, :],
                                wih_sb[:, k, j * 512:(j + 1) * 512],
                                start=(k == 0), stop=(k == KE - 1))
                        g32 = pop.tile([128, 512], F32, tag="g32")
                        nc.vector.tensor_add(g32[:], ps[:], bb_sb[:, d, j, :])
                        phi = pop.tile([128, 512], BF16, tag="phi")
                        nc.scalar.copy(phi[:], g32[:])
                        plo = pop.tile([128, 512], BF16, tag="plo")
                        nc.vector.tensor_sub(plo[:], g32[:], phi[:])
                        nc.sync.dma_start(pD[d, m, j, 0], phi[:])
                        nc.sync.dma_start(pD[d, m, j, 1], plo[:])

        # ---- phase 2: two interleaved sequential scans ----
        # Emission order per (t, d): MMs + elementwise chain for stream d,
        # then the hbar production of the PREVIOUS stream (whose chain ran
        # during this stream's matmuls) -- keeps the PE queue stall-free.
        with tc.tile_pool(name="ppf", bufs=3) as pfp, \
             tc.tile_pool(name="gsb", bufs=2) as gp, \
             tc.tile_pool(name="gps", bufs=2, space="PSUM") as gpsp, \
             tc.tile_pool(name="tps", bufs=1, space="PSUM") as tpsp:

            def emit_hbar(pending):
                pd_, t_p, h2_p = pending
                h2c = gp.tile([BL, KH, 128], F32, tag=f"h2c{pd_}")
                for j in range(NJ):
                    nc.gpsimd.tensor_copy(h2c[:, j, :], h2_p[32 * j:32 * j + BL, :])
                tp = tpsp.tile([128, KH, BL], F32, tag=f"tp{pd_}")
                for j in range(NJ):
                    nc.tensor.matmul(
                        tp[:, j, :], h2c[:, j, :], idt_sb[:],
                        start=True, stop=True, tile_position=(0, 0))
                nc.vector.tensor_copy(hTh[:, pd_, :, :, t_p + 1], tp[:])
                nc.vector.tensor_sub(hTl[:, pd_, :, :, t_p + 1], tp[:],
                                     hTh[:, pd_, :, :, t_p + 1])

            pending = None
            for t in range(S):
                for d in range(ND):
                    m, dt = divmod(t, TPM)
                    p_sb = pfp.tile([128, 512], BF16, tag=f"p{d}")
                    for j in range(NJ):
                        nc.sync.dma_start(
                            p_sb[32 * j:32 * j + 2 * BL, :],
                            pD[d, m, j, :, dt * BL:(dt + 1) * BL, :])
                    ps = gpsp.tile([128, 512], F32, tag=f"ps{d}")
                    for k in range(KH):
                        for j in range(NJ):
                            nc.tensor.matmul(
                                ps[32 * j:32 * j + BL, :],
                                hTh[:, d, k, :, t],
                                whh_sb[:, d, k, j * 512:(j + 1) * 512],
                                start=(k == 0), stop=False,
                                tile_position=(0, 32 * j))
                    # p_hi + p_lo accumulate via stacked [I4;I4] stationary
                    for j in range(NJ):
                        nc.tensor.matmul(
                            ps[32 * j:32 * j + BL, :],
                            idp_sb[32 * j:32 * j + 2 * BL, :],
                            p_sb[32 * j:32 * j + 2 * BL, :],
                            start=False, stop=True,
                            tile_position=(32 * j, 32 * j))
                    # tanh of all gates; sigmoid gates use tanh(z/2)
                    tan = gp.tile([128, 512], F32, tag=f"tan{d}")
                    nc.scalar.activation(tan[:], ps[:], AF.Tanh, scale=0.5)
                    ti, tf = tan[:, 0:128], tan[:, 128:256]
                    to, tg = tan[:, 256:384], tan[:, 384:512]
                    t1 = gp.tile([128, 128], F32, tag=f"t1{d}")
                    nc.vector.scalar_tensor_tensor(
                        t1[:], ti, 1.0, tg, ALU.add, ALU.mult)
                    t2 = gp.tile([128, 128], F32, tag=f"t2{d}")
                    nc.vector.scalar_tensor_tensor(
                        t2[:], tf, 1.0, c2[:, d], ALU.add, ALU.mult)
                    nc.vector.scalar_tensor_tensor(
                        c2[:, d], t2[:], 0.5, t1[:], ALU.mult, ALU.add)
                    tc_ = gp.tile([128, 128], F32, tag=f"tc{d}")
                    nc.scalar.activation(tc_[:], c2[:, d], AF.Tanh, scale=0.5)
                    h2 = gp.tile([128, 128], F32, tag=f"h2{d}")
                    nc.vector.scalar_tensor_tensor(
                        h2[:], to, 1.0, tc_[:], ALU.add, ALU.mult)
                    if pending is not None:
                        emit_hbar(pending)
                    pending = (d, t, h2)
            emit_hbar(pending)

        # ---- phase 3 (fp32): feats^T = Wo_perm^T @ (h_hi + h_lo) ----
        with tc.tile_pool(name="f3", bufs=2) as f3p, \
             tc.tile_pool(name="f3w", bufs=1) as f3w, \
             tc.tile_pool(name="f3ps", bufs=2, space="PSUM") as f3ps:
            wo_sb = f3w.tile([128, ND, KH, T], F32, tag="wo")
            nc.sync.dma_start(wo_sb[:], wo.rearrange("d k p t -> p d k t"))
            for d in range(ND):
                hTf = f3w.tile([128, KH, BL, S], F32, tag="hTf")
                for k in range(KH):
                    nc.vector.tensor_copy(hTf[:, k], hTh[:, d, k, :, 1:S + 1])
                    nc.vector.tensor_add(hTf[:, k], hTf[:, k],
                                         hTl[:, d, k, :, 1:S + 1])
                for b in range(BL):
                    ps = f3ps.tile([T, S], F32, tag="fps")
                    for k in range(KH):
                        nc.tensor.matmul(ps[:], wo_sb[:, d, k, :],
                                         hTf[:, k, b, :],
                                         start=(k == 0), stop=(k == KH - 1))
                    fo = f3p.tile([T, S], F32, tag="fo")
                    nc.scalar.copy(fo[:], ps[:])
                    nc.sync.dma_start(feats[d, b], fo[:])
    return nc


_NC_CACHE = {}


def _get_nc():
    if "nc" not in _NC_CACHE:
        nc = bacc.Bacc("TRN2")
        build_program(nc)
        nc.finalize()
        _NC_CACHE["nc"] = nc
    return _NC_CACHE["nc"]


def _gate_perm():
    """Column permutation: for hidden slice j, [i_j, f_j, o_j, g_j] (128 each)."""
    idx = []
    for j in range(NJ):
        sl = slice(128 * j, 128 * j + 128)
        idx += list(range(0 * H, 1 * H))[sl]      # i
        idx += list(range(1 * H, 2 * H))[sl]      # f
        idx += list(range(3 * H, 4 * H))[sl]      # o
        idx += list(range(2 * H, 3 * H))[sl]      # g
    return np.array(idx)


_PERM = _gate_perm()
_GMASK = np.zeros(G4, np.float32)
for _j in range(NJ):
    _GMASK[_j * 512 + 384:_j * 512 + 512] = 1.0  # g-columns after permute


def _prep_weights(Wih_f, Whh_f, b_f, Wih_b, Whh_b, b_b, W_out):
    """Returns per-direction permuted/scaled weight arrays (host)."""
    out = {}
    wihs, whhs, bbs, wos = [], [], [], []
    for d, (Wih, Whh, bv) in enumerate(
            [(Wih_f, Whh_f, b_f), (Wih_b, Whh_b, b_b)]):
        sc = (1.0 + _GMASK)                      # x2 on g columns
        wihT = np.asarray(Wih, np.float32).T[:, _PERM] * sc[None, :]
        whhT = np.asarray(Whh, np.float32).T[:, _PERM] * (0.5 * sc)[None, :]
        bp = np.asarray(bv, np.float32)[_PERM] * sc
        wihs.append(wihT.astype(np.float32))
        whhs.append(whhT.astype(np.float16))
        bbs.append(np.tile(bp.reshape(1, NJ, 512), (128, 1, 1)))
        wo_half = np.asarray(W_out, np.float32)[:, d * H:(d + 1) * H]
        woT = (wo_half.T * 0.5).reshape(KH, 128, T)
        wos.append(woT.astype(np.float32))
    out["wih"] = np.stack(wihs)
    out["whh"] = np.stack(whhs)
    out["bb"] = np.stack(bbs).astype(np.float32)
    out["wo"] = np.stack(wos)
    idp = np.zeros((128, BL), np.float32)
    for j in range(NJ):
        idp[32 * j:32 * j + BL] = np.eye(BL)
        idp[32 * j + BL:32 * j + 2 * BL] = np.eye(BL)
    out["identp"] = idp.astype(BF16NP)
    out["identt"] = np.eye(BL, dtype=np.float32)
    return out


def make_in_maps(emb, Wih_f, Whh_f, b_f, Wih_b, Whh_b, b_b, W_out):
    """emb: [B, S, E] float32. Returns per-core input maps."""
    wts = _prep_weights(Wih_f, Whh_f, b_f, Wih_b, Whh_b, b_b, W_out)
    in_maps = []
    for c in range(NCORES):
        x = emb[c * BL:(c + 1) * BL]                      # [BL, S, E]
        m = dict(wts)
        for d in range(ND):
            xd = x if d == 0 else x[:, ::-1]
            # [E, t*BL + b]
            m[f"xt{d}"] = np.ascontiguousarray(
                xd.transpose(2, 1, 0).reshape(E, S * BL)).astype(np.float32)
        in_maps.append(m)
    return in_maps


def assemble_feats(results, b_out):
    feats = np.zeros((B, S, T), np.float32)
    for c in range(NCORES):
        f = np.asarray(results[c]["feats"], np.float32)   # [ND, BL, T, S]
        ff = np.transpose(f[0], (0, 2, 1))                # [BL, S, T]
        fb = np.transpose(f[1], (0, 2, 1))[:, ::-1]
        feats[c * BL:(c + 1) * BL] = ff + fb
    feats += np.asarray(b_out, np.float32)[None, None, :]
    return feats


def viterbi(feats, trans, start, stop):
    Bq, Sq, Tq = feats.shape
    v = feats[:, 0] + start[None, :]
    idxs = np.zeros((Sq - 1, Bq, Tq), np.int32)
    for s in range(1, Sq):
        scores = v[:, :, None] + trans[None, :, :]
        idxs[s - 1] = np.argmax(scores, axis=1)
        v = np.max(scores, axis=1) + feats[:, s]
    last = np.argmax(v + stop[None, :], axis=-1).astype(np.int32)
    tags = np.zeros((Bq, Sq), np.int32)
    tags[:, -1] = last
    cur = last
    for s in range(Sq - 2, -1, -1):
        cur = idxs[s][np.arange(Bq), cur].astype(np.int32)
        tags[:, s] = cur
    return tags


def kernel(sentence, embedding, Wih_f, Whh_f, b_f, Wih_b, Whh_b, b_b,
           W_out, b_out, transitions, start_trans, stop_trans):
    sentence = np.asarray(sentence)
    emb = np.asarray(embedding, np.float32)[sentence.astype(np.int64)]
    nc = _get_nc()
    in_maps = make_in_maps(emb, np.asarray(Wih_f), np.asarray(Whh_f),
                           np.asarray(b_f), np.asarray(Wih_b),
                           np.asarray(Whh_b), np.asarray(b_b),
                           np.asarray(W_out))
    res = run_bass_kernel_spmd(nc, in_maps, list(range(NCORES))).results
    feats = assemble_feats(res, np.asarray(b_out))
    return viterbi(feats, np.asarray(transitions, np.float32),
                   np.asarray(start_trans, np.float32),
                   np.asarray(stop_trans, np.float32))
